# revision 57
# baseline (speedup 1.0000x reference)
"""Decision Transformer forward pass as a Bass/Tile kernel for 8 Trainium2 cores.

Sharding: data-parallel over batch (B=8 -> 1 element per core), no collectives.
All activations are feature-major (features on partitions, sequence on the free
dim); weights are pre-tiled on the host so every matmul is lhsT.T @ rhs with no
on-device transposes. Matmuls run as float32r; the attention data path
(probs/V/Y/Wp) is fp16 to fit SBUF.
"""
import sys

sys.path.insert(0, "/opt/trn_rl_repo")

from contextlib import ExitStack

import ml_dtypes
import numpy as np

import concourse.bass as bass
import concourse.tile as tile
from concourse import bacc, mybir
from concourse import bass_utils
from concourse.hw_specs import get_activation_tables

f32 = mybir.dt.float32
f16 = mybir.dt.float16
f32r = mybir.dt.float32r
f8 = mybir.dt.float8e4
DR = mybir.MatmulPerfMode.DoubleRow
AF = mybir.ActivationFunctionType
ALU = mybir.AluOpType
WS = 64.0            # fp8 weight pre-scale (keeps 0.02-scale weights normal)
WSI = 1.0 / WS

H = 768
FF = 3072
L = 4
NH = 12
HD = 64
B = 8
T = 64
M = 13
TEXT = 64
IMG_CH = 3
IMG_DIM = 49
ACT = 7
MAX_EP = 1024
STATE_TOK = IMG_CH + M          # 16
TOTAL = 2 + STATE_TOK           # 18
S = TOTAL * T                   # 1152
NEG = -10000.0
LN_EPS = 1e-5

KT = 6          # H/128
ST = 9          # S/128
CH = 384        # matmul free chunk
NCH = 3         # S/CH
FT = 24         # FF/128
MG = 6          # MLP ff-group size
N_CORES = 8
MISC_K = IMG_DIM + ACT + 1 + 3 + STATE_TOK   # 76


def bcast_free(ap_, reps):
    """Append a 0-stride free dim (broadcast read along free axis)."""
    return bass.AP(tensor=ap_.tensor, offset=ap_.offset, ap=list(ap_.ap) + [[0, reps]])


def build_kernel(pad_masked: bool, b2nz: bool = False, emb_triv: bool = True,
                 dbg: bool = False):
    nc = bacc.Bacc("TRN2", target_bir_lowering=False, debug=False)
    # Steer Ln/Exp table-set selection to the combined set (contains both
    # fns), so the ln->exp chains in softmax-recip/layernorm don't reload
    # ACT tables (1.3us each) on every alternation.
    tabs = get_activation_tables(nc.m.arch)
    tabs["exp_and_others"].discard(AF.Exp)
    tabs["natural_log"].discard(AF.Ln)

    def din(name, shape, dt=f32):
        return nc.dram_tensor(name, shape, dt, kind="ExternalInput").ap()

    misc_lhs = din("misc_lhs", (MISC_K, H))
    misc_rhs = din("misc_rhs", (MISC_K, S))
    mis_lhs = din("mis_lhs", (TEXT, H))
    mis_rhs = din("mis_rhs", (TEXT, S))
    time_lhs = din("time_lhs", (KT, 8, 128, 128), f16)   # [fo, kb, p, c]
    time_rhs = din("time_rhs", (8, 128, T), f16)         # [kb, p, t] one-hot
    lnemb_g = din("lnemb_g", (128, KT))
    lnemb_b = din("lnemb_b", (128, KT))
    wq_d = din("wq", (L, KT, KT, 128, 128), f16)              # [l, fo, kb, p, c]
    wk_d = din("wk", (L, KT, KT, 128, 128), f16)
    wp_d = din("wp", (L, KT, KT, 128, 128), f16)
    wv_d = din("wv", (L, KT, 128, H), f16)                    # [l, kb, p, c]
    w1_d = din("w1", (L, FT // MG, MG, KT, 128, 128), f16)    # [l, g, f, kb, p, c]
    w2_d = din("w2", (L, FT // MG, MG, KT, 128, 128), f16)    # [l, g, kb(FF), fo, p, c]
    bq_d = din("bq", (L, 128, KT))
    bk_d = din("bk", (L, 128, KT))
    bv_d = din("bv", (L, 1, H))
    bp_d = din("bp", (L, 128, KT))
    b1_d = din("b1", (L, 128, FT))
    b2_d = din("b2", (L, 128, KT))
    masks_d = din("masks", (128, 3, CH), f16)
    ident_d = din("ident", (128, 128), f16)
    ccol_d = din("const_col", (1, 128, 1))
    crow_d = din("const_row", (2, 1, 128))
    wpred_d = din("wpred", (128, KT, ACT))
    bpred_d = din("bpred", (ACT, 1))
    padm_d = din("padm", (128, ST)) if pad_masked else None
    out_d = nc.dram_tensor("out", (ACT, S), f32, kind="ExternalOutput").ap()
    if dbg:
        def dout(name, shape, dt=f32):
            return nc.dram_tensor(name, shape, dt, kind="ExternalOutput").ap()
        stk_d = dout("dbg_stk", (KT, 128, S))
        h0_d = dout("dbg_h0", (KT, 128, S))
        hn0_d = dout("dbg_hn0", (KT, 128, S), f16)
        v0_d = dout("dbg_v0", (ST, 128, NH * (HD + 1)), f16)
        qt0_d = dout("dbg_qt0", (128, S), f16)
        kt0_d = dout("dbg_kt0", (128, S), f16)
        att0_d = dout("dbg_att0", (128, ST * CH), f16)
        yt0_d = dout("dbg_yt0", (KT, 128, S), f16)
        hp0_d = dout("dbg_hp0", (KT, 128, S))
        hL0_d = dout("dbg_hL0", (KT, 128, S))

    with tile.TileContext(nc) as tc, ExitStack() as ctx:
        ctx.enter_context(
            nc.allow_low_precision(reason="f32r/f16 storage; psum accum is f32"))
        const = ctx.enter_context(tc.tile_pool(name="const", bufs=1))
        resid = ctx.enter_context(tc.tile_pool(name="resid", bufs=1))
        hnp = ctx.enter_context(tc.tile_pool(name="hnp", bufs=1))
        vpool = ctx.enter_context(tc.tile_pool(name="vpool", bufs=1))
        ytp = ctx.enter_context(tc.tile_pool(name="ytp", bufs=1))
        qkp = ctx.enter_context(tc.tile_pool(name="qkp", bufs=1))
        attp = ctx.enter_context(tc.tile_pool(name="attp", bufs=3))
        wvp = ctx.enter_context(tc.tile_pool(name="wvp", bufs=1))
        wstr = ctx.enter_context(tc.tile_pool(name="wstr", bufs=2))
        wgrp = ctx.enter_context(tc.tile_pool(name="wgrp", bufs=2))
        bstr = ctx.enter_context(tc.tile_pool(name="bstr", bufs=2))
        tmp = ctx.enter_context(tc.tile_pool(name="tmp", bufs=3))
        stats = ctx.enter_context(tc.tile_pool(name="stats", bufs=3))
        apool = ctx.enter_context(tc.tile_pool(name="apool", bufs=2))
        psp = ctx.enter_context(tc.tile_pool(name="psp", bufs=1, space="PSUM"))

        def ps1(p=128, n=CH):
            return psp.tile([p, n], f32, name="b1", tag="b1", bufs=4)

        def ps2():
            return psp.tile([128, 2, 512], f32, name="b2", tag="b2", bufs=2)

        ones_col = const.tile([128, 1], f32r, name="ones_col", tag="ones_col")
        nc.sync.dma_start(ones_col[:], ccol_d[0].bitcast(f32r))
        ones_row = const.tile([1, 128], f32r, name="ones_row", tag="ones_row")
        nc.sync.dma_start(ones_row[:], crow_d[0].bitcast(f32r))
        h768_row = const.tile([1, 128], f32r, name="h768_row", tag="h768_row")
        nc.sync.dma_start(h768_row[:], crow_d[1].bitcast(f32r))
        masks_sb = const.tile([128, 3, CH], f16, name="masks", tag="masks")
        nc.sync.dma_start(masks_sb[:], masks_d)
        ident_sb = const.tile([128, 128], f16, name="ident", tag="ident")
        nc.sync.dma_start(ident_sb[:], ident_d)
        g_sb = const.tile([128, KT], f32, name="g_sb", tag="g_sb")
        nc.sync.dma_start(g_sb[:], lnemb_g)
        gb_sb = const.tile([128, KT], f32, name="gb_sb", tag="gb_sb")
        nc.sync.dma_start(gb_sb[:], lnemb_b)
        wpred_sb = const.tile([128, KT, ACT], f32r, name="wpred", tag="wpred")
        nc.sync.dma_start(wpred_sb[:], wpred_d.bitcast(f32r))
        bpred_sb = const.tile([ACT, 1], f32, name="bpred", tag="bpred")
        nc.sync.dma_start(bpred_sb[:], bpred_d)
        eps_sb = const.tile([1, 1], f32, name="eps_sb", tag="eps_sb")
        nc.vector.memset(eps_sb[:], float(H) * float(H) * LN_EPS)
        if pad_masked:
            padm_sb = const.tile([128, ST], f32, name="padm", tag="padm")
            nc.sync.dma_start(padm_sb[:], padm_d)

        hT = [resid.tile([128, S], f32r, name=f"h{k}", tag=f"h{k}") for k in range(KT)]
        hn = [hnp.tile([128, S], f16, name=f"hn{k}", tag=f"hn{k}") for k in range(KT)]
        yt = [ytp.tile([128, S], f16, name=f"y{k}", tag=f"y{k}") for k in range(KT)]

        def chunk(c):
            return slice(c * CH, (c + 1) * CH)

        def pe_warm(n):
            # no-reader identity matmuls: queued where the PE would
            # otherwise idle >3.4us on DVE/ACT chains, they hold the HAM
            # clock gate at 8/8 through the embed phase
            pw = ps1(128, 128)
            for _ in range(n):
                nc.tensor.matmul(pw[:], ident_sb[:], ident_sb[:],
                                 start=True, stop=True)


        def layernorm_chunk(xs, outs, c, gain=None, gbias=None, fp8_copy=False):
            # stats for chunk c, then normalize chunk c; emitting per-chunk
            # lets chunk c's DVE/ACT tail overlap chunk c+1's PE stats work
            sl = chunk(c)
            ps_s = ps1(1)
            ps_q = ps1(1)
            for k in range(KT):
                nc.tensor.matmul(ps_s[:], ones_col[:], xs[k][:, sl],
                                 start=(k == 0), stop=(k == KT - 1))
            for k in range(KT):
                sq = tmp.tile([128, CH], f32r, name="tm", tag="tm")
                nc.scalar.activation(sq[:], xs[k][:, sl].bitcast(f32), AF.Square)
                nc.tensor.matmul(ps_q[:], ones_col[:], sq[:],
                                 start=(k == 0), stop=(k == KT - 1))
            sums = stats.tile([1, CH], f32, name="st", tag="st")
            nc.vector.tensor_copy(sums[:], ps_s[:])
            t0 = stats.tile([1, CH], f32, name="st", tag="st")
            nc.vector.tensor_mul(t0[:], sums[:], sums[:])
            vx = stats.tile([1, CH], f32, name="st", tag="st")
            nc.vector.scalar_tensor_tensor(
                out=vx[:], in0=ps_q[:], scalar=float(H), in1=t0[:],
                op0=ALU.mult, op1=ALU.subtract)
            lnv = stats.tile([1, CH], f32, name="st", tag="st")
            nc.scalar.activation(lnv[:], vx[:], AF.Ln, bias=eps_sb[:])
            r_ = stats.tile([1, CH], f32r, name="rst", tag="rst", bufs=3)
            nc.scalar.activation(r_[:], lnv[:], AF.Exp, scale=-0.5)
            bv_ = stats.tile([1, CH], f32r, name="rst", tag="rst", bufs=3)
            nc.vector.scalar_tensor_tensor(
                out=bv_[:], in0=sums[:], scalar=-1.0, in1=r_[:],
                op0=ALU.mult, op1=ALU.mult)
            pa = ps1()
            nc.tensor.matmul(pa[:], h768_row[:], r_[:], start=True, stop=True)
            pb = ps1()
            nc.tensor.matmul(pb[:], ones_row[:], bv_[:], start=True, stop=True)
            for k in range(KT):
                tm = tmp.tile([128, CH], f32, name="tm", tag="tm")
                nc.vector.tensor_mul(tm[:], xs[k][:, sl], pa[:])
                if gain is None:
                    nc.vector.tensor_add(outs[k][:, sl], tm[:], pb[:])
                else:
                    tm2 = tmp.tile([128, CH], f32, name="tm", tag="tm")
                    nc.vector.tensor_add(tm2[:], tm[:], pb[:])
                    nc.vector.tensor_scalar(
                        out=outs[k][:, sl], in0=tm2[:],
                        scalar1=gain[:, k:k + 1], scalar2=gbias[:, k:k + 1],
                        op0=ALU.mult, op1=ALU.add)

        def layernorm(xs, outs, gain=None, gbias=None, fp8_copy=False):
            for c in range(NCH):
                layernorm_chunk(xs, outs, c, gain=gain, gbias=gbias,
                                fp8_copy=fp8_copy)

        # ================= embedding =================
        # scratch tiles share the weight-group slots (dead after embedding)
        sc_a = wgrp.tile([128, 2304], f32r, name="w1g", tag="w1g")
        sc_b = wgrp.tile([128, 2304], f32r, name="w2g", tag="w2g")
        misc_l = sc_a[0:MISC_K, 0:H]
        nc.sync.dma_start(misc_l, misc_lhs.bitcast(f32r))
        mis_l = sc_a[0:TEXT, H:H + H]
        nc.sync.dma_start(mis_l, mis_lhs.bitcast(f32r))
        misc_r = sc_b[0:MISC_K, 0:S]
        nc.sync.dma_start(misc_r, misc_rhs.bitcast(f32r))
        mis_r = sc_b[0:TEXT, S:2 * S]
        nc.sync.dma_start(mis_r, mis_rhs.bitcast(f32r))
        trhs = const.tile([128, 8, T], f16, name="trhs", tag="trhs")
        nc.sync.dma_start(trhs[:], time_rhs.rearrange("kb p t -> p kb t"))

        stk = [hnp.tile([128, S], f32r, name=f"stk{k}", tag=f"hn{k}")
               for k in range(KT)]
        if emb_triv:
            pe_warm(40)  # shares the hn slots
        for k in range(KT):
            fsl = slice(k * 128, (k + 1) * 128)
            tl = wstr.tile([128, 8, 128], f16, name="wq", tag="wq")
            nc.sync.dma_start(tl[:], time_lhs[k].rearrange("kb p c -> p kb c"))
            ptime = ps1(128, T)
            for kb in range(8):
                nc.tensor.matmul(ptime[:], tl[:, kb, :], trhs[:, kb, :],
                                 start=(kb == 0), stop=(kb == 7))
            tsb = tmp.tile([128, T], f32, name="tm", tag="tm")
            nc.vector.tensor_copy(tsb[:], ptime[:])
            for c in range(NCH):
                sl = chunk(c)
                pstk = ps1()
                nc.tensor.matmul(pstk[:], misc_l[:, fsl], misc_r[:, sl],
                                 start=True, stop=False)
                nc.tensor.matmul(pstk[:], mis_l[:, fsl], mis_r[:, sl],
                                 start=False, stop=True)
                nc.vector.tensor_copy(stk[k][:, sl], pstk[:])
            stk_v = stk[k][:].rearrange("p (t a) -> p t a", a=TOTAL)
            nc.vector.tensor_tensor(out=stk_v, in0=stk_v,
                                    in1=bcast_free(tsb[:], TOTAL), op=ALU.add)
        if dbg:
            for k in range(KT):
                nc.sync.dma_start(stk_d[k].bitcast(f32r), stk[k][:])
        if emb_triv:
            # ln_emb gamma==1/beta==0: use the cheaper no-gain path, and
            # LN(LN(x)) == LN(x) to within eps/2 (~5e-6), so layer 0's
            # attention layernorm collapses to a cast copy
            for c in range(NCH):
                pe_warm(30)
                layernorm_chunk(stk, hT, c)
            pe_warm(30)
            for k in range(KT):
                nc.vector.tensor_copy(hn[k][:], hT[k][:])
        else:
            layernorm(stk, hT, gain=g_sb, gbias=gb_sb)
            layernorm(hT, hn)   # LN1 for layer 0
        if dbg:
            for k in range(KT):
                nc.sync.dma_start(h0_d[k].bitcast(f32r), hT[k][:])

        # ================= layers =================
        for l in range(L):
            if dbg and l == 0:
                for k in range(KT):
                    nc.sync.dma_start(hn0_d[k], hn[k][:])
            bq_sb = bstr.tile([128, KT], f32, name="bq", tag="bq")
            nc.sync.dma_start(bq_sb[:], bq_d[l])
            bk_sb = bstr.tile([128, KT], f32, name="bk", tag="bk")
            nc.sync.dma_start(bk_sb[:], bk_d[l])
            bp_sb = bstr.tile([128, KT], f32, name="bp", tag="bp")
            nc.sync.dma_start(bp_sb[:], bp_d[l])
            b1_sb = bstr.tile([128, FT], f32, name="b1", tag="b1")
            nc.sync.dma_start(b1_sb[:], b1_d[l])
            b2_sb = bstr.tile([128, KT], f32, name="b2", tag="b2")
            nc.sync.dma_start(b2_sb[:], b2_d[l])
            bvb = qkp.tile([128, H], f32, name="bvb", tag="bvb")
            nc.sync.dma_start(bvb[:], bass.AP(tensor=bv_d.tensor,
                                              offset=bv_d[l].offset,
                                              ap=[[0, 128], [1, H]]))

            # ---- V (seq-major, fp16, ones column per head for softmax den) ----
            wv_sb = []
            for kb in range(KT):
                w_ = wvp.tile([128, H], f16, name=f"wvp{kb}", tag=f"wvp{kb}")
                nc.sync.dma_start(w_[:], wv_d[l, kb])
                wv_sb.append(w_)
            vt = []
            for si in range(ST):
                v_ = vpool.tile([128, NH, HD + 1], f16, name=f"v{si}", tag=f"v{si}")
                vt.append(v_)
                nc.vector.memset(v_[:, :, HD:HD + 1], 1.0)
            for si in range(ST):
                ssl = slice(si * 128, (si + 1) * 128)
                for c in range(2):
                    csl = slice(c * CH, (c + 1) * CH)
                    pv = ps1()
                    for kb in range(KT):
                        nc.tensor.matmul(pv[:], hn[kb][:, ssl], wv_sb[kb][:, csl],
                                         start=(kb == 0), stop=(kb == KT - 1))
                    dst = vt[si][:, c * KT:(c + 1) * KT, 0:HD]
                    nc.vector.tensor_tensor(
                        out=dst, in0=pv[:].rearrange("p (h d) -> p h d", d=HD),
                        in1=bvb[:, csl].rearrange("p (h d) -> p h d", d=HD),
                        op=ALU.add)

            if dbg and l == 0:
                for si in range(ST):
                    nc.sync.dma_start(
                        v0_d[si], vt[si][:].rearrange("p h d -> p (h d)"))

            # ---- attention: per-pair Q/K, then a software-pipelined stream of
            # (head, chunk) units: scores+exp now, AV one unit later,
            # normalize two units later (hides the reciprocal latency) ----
            wp_sb = []
            for fo in range(KT):
                w_ = wvp.tile([128, KT, 128], f16, name=f"wvp{fo}", tag=f"wvp{fo}")
                wp_sb.append(w_)
            qkt = {}

            def emit_qk(p):
                wq_sb = wstr.tile([128, KT, 128], f16, name="wq", tag="wq")
                nc.sync.dma_start(
                    wq_sb[:],
                    wq_d[l, p].rearrange("kb p c -> p kb c"))
                wk_sb = wstr.tile([128, KT, 128], f16, name="wk", tag="wk")
                nc.sync.dma_start(
                    wk_sb[:],
                    wk_d[l, p].rearrange("kb p c -> p kb c"))
                qt = qkp.tile([128, S], f16, name="qt", tag="qt")
                kt = qkp.tile([128, S], f16, name="kt", tag="kt")
                for c in range(NCH):
                    sl = chunk(c)
                    pq = ps1()
                    for kb in range(KT):
                        nc.tensor.matmul(pq[:], wq_sb[:, kb, :], hn[kb][:, sl],
                                         start=(kb == 0), stop=(kb == KT - 1))
                    nc.vector.tensor_scalar_add(out=qt[:, sl], in0=pq[:],
                                                scalar1=bq_sb[:, p:p + 1])
                    pk = ps1()
                    for kb in range(KT):
                        nc.tensor.matmul(pk[:], wk_sb[:, kb, :], hn[kb][:, sl],
                                         start=(kb == 0), stop=(kb == KT - 1))
                    nc.vector.tensor_scalar_add(out=kt[:, sl], in0=pk[:],
                                                scalar1=bk_sb[:, p:p + 1])
                if dbg and l == 0 and p == 0:
                    nc.sync.dma_start(qt0_d, qt[:])
                    nc.sync.dma_start(kt0_d, kt[:])
                # Wp column block for this pair (reuses the wv slots)
                nc.sync.dma_start(wp_sb[p][:],
                                  wp_d[l, p].rearrange("kb p c -> p kb c"))
                qkt[p] = (qt, kt)

            def emit_scores(u):
                p, hi, c = u["p"], u["hi"], u["c"]
                qt, kt = qkt[p]
                hsl = slice(hi * HD, (hi + 1) * HD)
                sl = chunk(c)
                nsk = 3 * (c + 1)
                att = attp.tile([128, ST, CH], f16, name="att", tag="att")
                for gs in range(0, nsk, 2):
                    ge = min(gs + 2, nsk)
                    psc = ps2()
                    for i in range(gs, ge):
                        diag = i - 3 * c
                        nc.tensor.matmul(psc[:, i - gs, 0:CH],
                                         kt[hsl, i * 128:(i + 1) * 128],
                                         qt[hsl, sl], start=True,
                                         stop=(diag < 0))
                        if diag >= 0:
                            # causal mask folded in on the PE: += -1e4
                            # above the diagonal (keeps exp->AV off the DVE)
                            nc.tensor.matmul(psc[:, i - gs, 0:CH],
                                             ident_sb[:],
                                             masks_sb[:, diag, :],
                                             start=False, stop=True)
                    if pad_masked:
                        for i in range(gs, ge):
                            nc.scalar.activation(att[:, i, :],
                                                 psc[:, i - gs, 0:CH],
                                                 AF.Exp, scale=0.125,
                                                 bias=padm_sb[:, i:i + 1])
                    else:
                        nc.scalar.activation(att[:, gs:ge, :],
                                             psc[:, 0:ge - gs, 0:CH],
                                             AF.Exp, scale=0.125)
                u["att"], u["hsl"], u["sl"], u["nsk"], u["h"] = \
                    att, hsl, sl, nsk, 2 * p + hi
                if dbg and l == 0 and u["h"] == 0 and c == 2:
                    nc.sync.dma_start(att0_d,
                                      att[:].rearrange("p i c -> p (i c)"))

            def emit_av(u):
                # AV into PSUM, then immediately evacuate: unnormalized Y
                # (f16) into yt, denominator row into the head's den batch.
                pY = ps1(HD + 1)
                for i in range(u["nsk"]):
                    nc.tensor.matmul(pY[:], vt[i][:, u["h"], :],
                                     u["att"][:, i, :],
                                     start=(i == 0), stop=(i == u["nsk"] - 1))
                nc.vector.tensor_copy(yt[u["p"]][u["hsl"], u["sl"]],
                                      pY[0:HD, :])
                nc.vector.tensor_copy(
                    u["den"][0:1, u["c"] * CH:(u["c"] + 1) * CH],
                    pY[HD:HD + 1, :])

            def emit_recip(units):
                # one Ln+Exp over the head's 3 chunk-denominators (batched
                # ACT; ln/exp share a table set so no ACT_TABLE_LOADs)
                den = units[0]["den"]
                lnd = stats.tile([1, S], f32, name="lnd", tag="den", bufs=2)
                nc.scalar.activation(lnd[:], den[:].bitcast(f32), AF.Ln)
                rec = stats.tile([1, S], f32r, name="rec", tag="rec", bufs=2)
                nc.scalar.activation(rec[:], lnd[:], AF.Exp, scale=-1.0)
                for u in units:
                    u["rec"] = rec

            def emit_norm(u):
                # broadcast recip to all 128 partitions so the in-place
                # multiply's SBUF operands share a start partition
                base = u["hi"] * HD
                pB = ps1()
                nc.tensor.matmul(pB[:], ones_row[0:1, :],
                                 u["rec"][0:1, u["sl"]], start=True, stop=True)
                yv = yt[u["p"]][u["hsl"], u["sl"]]
                nc.vector.tensor_tensor(out=yv, in0=yv,
                                        in1=pB[base:base + HD, :], op=ALU.mult)

            av_q = []
            done_heads = []

            def flush_av():
                u = av_q.pop(0)
                emit_av(u)
                if u["p"] == KT - 1 and u["hi"] == 1:
                    # final head: per-chunk in-place recip so the layer's
                    # drain tail stays under the HAM re-throttle window
                    units = u["units"]
                    if u["c"] == 0:
                        units[0]["recx"] = stats.tile(
                            [1, S], f32r, name="rec", tag="rec", bufs=2)
                    rec = units[0]["recx"]
                    csl = slice(u["c"] * CH, (u["c"] + 1) * CH)
                    nc.scalar.activation(rec[0:1, csl],
                                         u["den"][0:1, csl].bitcast(f32),
                                         AF.Ln)
                    nc.scalar.activation(rec[0:1, csl],
                                         rec[0:1, csl].bitcast(f32),
                                         AF.Exp, scale=-1.0)
                    u["rec"] = rec
                    if u["c"] == NCH - 1:
                        done_heads.append(units)
                        if len(done_heads) > 1:
                            for v in done_heads.pop(0):
                                emit_norm(v)
                elif u["c"] == NCH - 1:
                    # head complete: recip it, norm the head before it
                    # (PE stays busy on newer heads' score matmuls)
                    emit_recip(u["units"])
                    done_heads.append(u["units"])
                    if len(done_heads) > 1:
                        for v in done_heads.pop(0):
                            emit_norm(v)

            for p in range(KT):
                emit_qk(p)
                for hi in range(2):
                    den = stats.tile([1, S], f32r, name="den", tag="den",
                                     bufs=2)
                    units = []
                    for c in range(NCH):
                        u = {"p": p, "hi": hi, "c": c, "den": den,
                             "units": units}
                        emit_scores(u)
                        units.append(u)
                        av_q.append(u)
                        if len(av_q) > 2:
                            flush_av()
            while av_q:
                flush_av()
            for units in done_heads:
                for v in units:
                    emit_norm(v)

            if dbg and l == 0:
                for k in range(KT):
                    nc.sync.dma_start(yt0_d[k], yt[k][:])
            # ---- proj + residual, with the pre-MLP layernorm interleaved
            # per chunk (chunk c's LN DVE tail overlaps chunk c+1's proj MMs)
            for c in range(NCH):
                sl = chunk(c)
                for fo in range(KT):
                    pp = ps1()
                    for kb in range(KT):
                        nc.tensor.matmul(pp[:], wp_sb[fo][:, kb, :], yt[kb][:, sl],
                                         start=(kb == 0), stop=(kb == KT - 1))
                    nc.vector.scalar_tensor_tensor(
                        out=hT[fo][:, sl], in0=pp[:], scalar=bp_sb[:, fo:fo + 1],
                        in1=hT[fo][:, sl], op0=ALU.add, op1=ALU.add)
                layernorm_chunk(hT, hn, c)

            if dbg and l == 0:
                for k in range(KT):
                    nc.sync.dma_start(hp0_d[k].bitcast(f32r), hT[k][:])
            # ---- MLP (stream W1/W2 in ff-groups, accumulate into hT) ----
            for g in range(FT // MG):
                w1g = wgrp.tile([128, MG, KT, 128], f16, name="w1g", tag="w1g")
                nc.sync.dma_start(
                    w1g[:],
                    w1_d[l, g].rearrange("f kb p c -> p f kb c"))
                w2g = wgrp.tile([128, MG, KT, 128], f16, name="w2g", tag="w2g")
                nc.sync.dma_start(
                    w2g[:],
                    w2_d[l, g].rearrange("f kb p c -> p f kb c"))
                for c in range(NCH):
                    sl = chunk(c)
                    py_a = ps2()
                    py_b = ps2()
                    py_c = ps1()
                    py_d = ps1()

                    def ypsum(fo):
                        if fo < 2:
                            return py_a[:, fo, 0:CH]
                        if fo < 4:
                            return py_b[:, fo - 2, 0:CH]
                        return (py_c if fo == 4 else py_d)[:]

                    for f in range(MG):
                        ff = g * MG + f
                        pa = ps1()
                        for kb in range(KT):
                            nc.tensor.matmul(pa[:], w1g[:, f, kb, :],
                                             hn[kb][:, sl],
                                             start=(kb == 0), stop=(kb == KT - 1))
                        aT = apool.tile([128, CH], f16, name="aT", tag="aT")
                        nc.scalar.activation(aT[:], pa[:], AF.Gelu,
                                             bias=b1_sb[:, ff:ff + 1])
                        for fo in range(KT):
                            nc.tensor.matmul(ypsum(fo), w2g[:, f, fo, :], aT[:],
                                             start=(f == 0), stop=(f == MG - 1))
                    last = (g == FT // MG - 1)
                    for fo in range(KT):
                        if last:
                            nc.vector.scalar_tensor_tensor(
                                out=hT[fo][:, sl], in0=ypsum(fo),
                                scalar=b2_sb[:, fo:fo + 1], in1=hT[fo][:, sl],
                                op0=ALU.add, op1=ALU.add)
                        else:
                            nc.vector.tensor_add(hT[fo][:, sl], ypsum(fo),
                                                 hT[fo][:, sl])
                    if last and l < L - 1:
                        # next layer's attention layernorm, chunk-interleaved
                        # into the last MLP group so hn chunk 0 is ready the
                        # moment the MLP matmuls finish (keeps PE warm into
                        # the next layer's V matmuls)
                        layernorm_chunk(hT, hn, c)
            if dbg and l == 0:
                for k in range(KT):
                    nc.sync.dma_start(hL0_d[k].bitcast(f32r), hT[k][:])
        # ================= prediction head =================
        pred_sb = hnp.tile([ACT, S], f32, name="hn0", tag="hn0")
        for c in range(NCH):
            sl = chunk(c)
            pp = ps1(ACT)
            for kb in range(KT):
                nc.tensor.matmul(pp[:], wpred_sb[:, kb, :], hT[kb][:, sl],
                                 start=(kb == 0), stop=(kb == KT - 1))
            nc.vector.tensor_scalar_add(out=pred_sb[:, sl], in0=pp[:],
                                        scalar1=bpred_sb[:])
        nc.sync.dma_start(out_d, pred_sb[:])

    nc.compile()
    return nc


# ---------------- host-side preparation ----------------

def _tile_lhs(w):
    """(Hin, Hout) -> [fo, kb, 128, 128] with [fo,kb,p,c] = w[128kb+p, 128fo+c]."""
    hin, hout = w.shape
    return np.ascontiguousarray(
        w.reshape(hin // 128, 128, hout // 128, 128).transpose(2, 0, 1, 3))


def prep_shared(inp):
    """Weight-side arrays, identical for every core."""
    d = {}
    g1, b1l = inp["ln1_g"], inp["ln1_b"]
    g2, b2l = inp["ln2_g"], inp["ln2_b"]
    wq = inp["Wq"] * g1[:, :, None]
    wk = inp["Wk"] * g1[:, :, None]
    wv = inp["Wv"] * g1[:, :, None]
    w1 = inp["W1"] * g2[:, :, None]
    bq = inp["bq"] + np.einsum("lh,lho->lo", b1l, inp["Wq"])
    bk = inp["bk"] + np.einsum("lh,lho->lo", b1l, inp["Wk"])
    bv = inp["bv"] + np.einsum("lh,lho->lo", b1l, inp["Wv"])
    b1 = inp["b1"] + np.einsum("lh,lho->lo", b2l, inp["W1"])
    d["wq"] = np.stack([_tile_lhs(wq[l]) for l in range(L)]).astype(np.float16)
    d["wk"] = np.stack([_tile_lhs(wk[l]) for l in range(L)]).astype(np.float16)
    d["wp"] = np.stack([_tile_lhs(inp["Wp"][l]) for l in range(L)]).astype(np.float16)
    d["wv"] = np.stack([wv[l].reshape(KT, 128, H) for l in range(L)]).astype(np.float16)
    w1t = np.stack([_tile_lhs(w1[l]) for l in range(L)])          # (L,24,6,128,128)
    d["w1"] = w1t.reshape(L, FT // MG, MG, KT, 128, 128).astype(np.float16)
    w2t = np.stack([_tile_lhs(inp["W2"][l]) for l in range(L)])   # (L,6,24,128,128)
    d["w2"] = np.ascontiguousarray(w2t.transpose(0, 2, 1, 3, 4)).reshape(
        L, FT // MG, MG, KT, 128, 128).astype(np.float16)
    d["bq"] = np.ascontiguousarray(bq.reshape(L, KT, 128).transpose(0, 2, 1))
    d["bk"] = np.ascontiguousarray(bk.reshape(L, KT, 128).transpose(0, 2, 1))
    d["bv"] = bv.reshape(L, 1, H)
    d["bp"] = np.ascontiguousarray(inp["bp"].reshape(L, KT, 128).transpose(0, 2, 1))
    d["b1"] = np.ascontiguousarray(b1.reshape(L, FT, 128).transpose(0, 2, 1))
    d["b2"] = np.ascontiguousarray(inp["b2"].reshape(L, KT, 128).transpose(0, 2, 1))
    # embedding misc lhs: [W_img | W_act | W_ret | b_img;b_act;b_ret | W_pos[:16]]
    d["misc_lhs"] = np.concatenate([
        inp["W_img"], inp["W_act"], inp["W_ret"],
        inp["b_img"][None], inp["b_act"][None], inp["b_ret"][None],
        inp["W_pos"][:STATE_TOK]], axis=0).astype(np.float32)
    d["mis_lhs"] = inp["W_mis"].astype(np.float32)
    d["time_lhs"] = _tile_lhs(inp["W_time"]).astype(np.float16)
    d["lnemb_g"] = np.ascontiguousarray(
        inp["ln_emb_g"].reshape(KT, 128).T).astype(np.float32)
    d["lnemb_b"] = np.ascontiguousarray(
        inp["ln_emb_b"].reshape(KT, 128).T).astype(np.float32)
    r = np.arange(128)[:, None]
    cc = np.arange(CH)[None, :]
    d["masks"] = np.stack([(cc < 128 * dd + r) * NEG for dd in range(3)],
                          axis=1).astype(np.float16)
    d["ident"] = np.eye(128, dtype=np.float16)
    d["wpred"] = np.ascontiguousarray(
        inp["W_pred"].reshape(KT, 128, ACT).transpose(1, 0, 2)).astype(np.float32)
    d["bpred"] = inp["b_pred"].reshape(ACT, 1).astype(np.float32)
    d["const_col"] = np.ones((1, 128, 1), np.float32)
    d["const_row"] = np.stack([np.ones((1, 128), np.float32),
                               np.full((1, 128), float(H), np.float32)])
    return {k: np.ascontiguousarray(v) for k, v in d.items()}


def prep_core(inp, b, pad_masked):
    """Per-core (per batch element) data tensors."""
    d = {}
    misc_rhs = np.zeros((MISC_K, S), np.float32)
    mis_rhs = np.zeros((TEXT, S), np.float32)
    timeoh = np.zeros((MAX_EP, T), np.float32)
    img = np.asarray(inp["images"][b], np.float32).reshape(T, IMG_DIM, IMG_CH)
    act = np.asarray(inp["actions"][b], np.float32)
    rtg = np.asarray(inp["returns_to_go"][b], np.float32)
    mis = np.asarray(inp["missions"][b])
    ts = np.asarray(inp["timesteps"][b])
    for t in range(T):
        base = TOTAL * t
        for c in range(IMG_CH):
            misc_rhs[0:IMG_DIM, base + 1 + c] = img[t, :, c]
        misc_rhs[IMG_DIM:IMG_DIM + ACT, base + TOTAL - 1] = act[t]
        misc_rhs[IMG_DIM + ACT, base] = rtg[t, 0]
        misc_rhs[IMG_DIM + ACT + 1, base + 1:base + 1 + IMG_CH] = 1.0
        misc_rhs[IMG_DIM + ACT + 2, base + TOTAL - 1] = 1.0
        misc_rhs[IMG_DIM + ACT + 3, base] = 1.0
        for j in range(STATE_TOK):
            misc_rhs[IMG_DIM + ACT + 4 + j, base + 1 + j] = 1.0
        for j in range(M):
            mis_rhs[mis[t, j], base + 1 + IMG_CH + j] = 1.0
        timeoh[ts[t], t] = 1.0
    d["misc_rhs"] = misc_rhs
    d["mis_rhs"] = mis_rhs
    d["time_rhs"] = timeoh.reshape(8, 128, T).astype(np.float16)
    if pad_masked:
        am = np.asarray(inp["attention_mask"][b], np.float32)
        mm = np.asarray(inp["mission_masks"][b], np.float32)
        tok = np.concatenate([np.ones((T, 1 + IMG_CH), np.float32), mm,
                              np.ones((T, 1), np.float32)], axis=1)
        m = (tok * am[:, None]).reshape(S)
        d["padm"] = np.ascontiguousarray(
            ((1.0 - m) * NEG).reshape(ST, 128).T).astype(np.float32)
    return d


_CACHE = {}


def _get_nc(pad_masked, b2nz=False, emb_triv=True):
    key = (pad_masked, b2nz, emb_triv)
    if key not in _CACHE:
        _CACHE[key] = build_kernel(pad_masked, b2nz, emb_triv)
    return _CACHE[key]


def kernel(**inputs):
    pad_masked = not (
        np.all(np.asarray(inputs["mission_masks"]) == 1.0)
        and np.all(np.asarray(inputs["attention_mask"]) == 1))
    b2nz = bool(np.any(np.asarray(inputs["b2"]) != 0))
    emb_triv = bool(np.all(np.asarray(inputs["ln_emb_g"]) == 1.0)
                    and np.all(np.asarray(inputs["ln_emb_b"]) == 0.0))
    nc = _get_nc(pad_masked, b2nz, emb_triv)
    shared = prep_shared({k: np.asarray(v) for k, v in inputs.items()})
    in_maps = []
    for b in range(N_CORES):
        m = dict(shared)
        m.update(prep_core(inputs, b, pad_masked))
        in_maps.append(m)
    res = bass_utils.run_bass_kernel_spmd(nc, in_maps, core_ids=list(range(N_CORES)))
    # gather: out[b, t] = predT_b[:, 18*t + valid_t + 3]
    mm = np.asarray(inputs["mission_masks"])
    out = np.zeros((B, T, ACT), np.float32)
    tt = np.arange(T)
    for b in range(B):
        gcol = TOTAL * tt + (mm[b].sum(axis=1) + IMG_CH).astype(np.int64)
        out[b] = np.asarray(res.results[b]["out"], np.float32)[:, gcol].T
    return out



# revision 58
# speedup vs baseline: 1.0038x; 1.0038x over previous
"""Decision Transformer forward pass as a Bass/Tile kernel for 8 Trainium2 cores.

Sharding: data-parallel over batch (B=8 -> 1 element per core), no collectives.
All activations are feature-major (features on partitions, sequence on the free
dim); weights are pre-tiled on the host so every matmul is lhsT.T @ rhs with no
on-device transposes. Matmuls run as float32r; the attention data path
(probs/V/Y/Wp) is fp16 to fit SBUF.
"""
import sys

sys.path.insert(0, "/opt/trn_rl_repo")

from contextlib import ExitStack

import ml_dtypes
import numpy as np

import concourse.bass as bass
import concourse.tile as tile
from concourse import bacc, mybir
from concourse import bass_utils
from concourse.hw_specs import get_activation_tables

f32 = mybir.dt.float32
f16 = mybir.dt.float16
f32r = mybir.dt.float32r
f8 = mybir.dt.float8e4
DR = mybir.MatmulPerfMode.DoubleRow
AF = mybir.ActivationFunctionType
ALU = mybir.AluOpType
WS = 64.0            # fp8 weight pre-scale (keeps 0.02-scale weights normal)
WSI = 1.0 / WS

H = 768
FF = 3072
L = 4
NH = 12
HD = 64
B = 8
T = 64
M = 13
TEXT = 64
IMG_CH = 3
IMG_DIM = 49
ACT = 7
MAX_EP = 1024
STATE_TOK = IMG_CH + M          # 16
TOTAL = 2 + STATE_TOK           # 18
S = TOTAL * T                   # 1152
NEG = -10000.0
LN_EPS = 1e-5

KT = 6          # H/128
ST = 9          # S/128
CH = 384        # matmul free chunk
NCH = 3         # S/CH
FT = 24         # FF/128
MG = 6          # MLP ff-group size
N_CORES = 8
MISC_K = IMG_DIM + ACT + 1 + 3 + STATE_TOK   # 76


def bcast_free(ap_, reps):
    """Append a 0-stride free dim (broadcast read along free axis)."""
    return bass.AP(tensor=ap_.tensor, offset=ap_.offset, ap=list(ap_.ap) + [[0, reps]])


def build_kernel(pad_masked: bool, b2nz: bool = False, emb_triv: bool = True,
                 dbg: bool = False):
    nc = bacc.Bacc("TRN2", target_bir_lowering=False, debug=False)
    # Steer Ln/Exp table-set selection to the combined set (contains both
    # fns), so the ln->exp chains in softmax-recip/layernorm don't reload
    # ACT tables (1.3us each) on every alternation.
    tabs = get_activation_tables(nc.m.arch)
    tabs["exp_and_others"].discard(AF.Exp)
    tabs["natural_log"].discard(AF.Ln)

    def din(name, shape, dt=f32):
        return nc.dram_tensor(name, shape, dt, kind="ExternalInput").ap()

    misc_lhs = din("misc_lhs", (MISC_K, H))
    misc_rhs = din("misc_rhs", (MISC_K, S))
    mis_lhs = din("mis_lhs", (TEXT, H))
    mis_rhs = din("mis_rhs", (TEXT, S))
    time_lhs = din("time_lhs", (KT, 8, 128, 128), f16)   # [fo, kb, p, c]
    time_rhs = din("time_rhs", (8, 128, T), f16)         # [kb, p, t] one-hot
    lnemb_g = din("lnemb_g", (128, KT))
    lnemb_b = din("lnemb_b", (128, KT))
    wq_d = din("wq", (L, KT, KT, 128, 128), f16)              # [l, fo, kb, p, c]
    wk_d = din("wk", (L, KT, KT, 128, 128), f16)
    wp_d = din("wp", (L, KT, KT, 128, 128), f16)
    wv_d = din("wv", (L, KT, 128, H), f16)                    # [l, kb, p, c]
    w1_d = din("w1", (L, FT // MG, MG, KT, 128, 128), f16)    # [l, g, f, kb, p, c]
    w2_d = din("w2", (L, FT // MG, MG, KT, 128, 128), f16)    # [l, g, kb(FF), fo, p, c]
    bq_d = din("bq", (L, 128, KT))
    bk_d = din("bk", (L, 128, KT))
    bv_d = din("bv", (L, 1, H))
    bp_d = din("bp", (L, 128, KT))
    b1_d = din("b1", (L, 128, FT))
    b2_d = din("b2", (L, 128, KT))
    masks_d = din("masks", (128, 3, CH), f16)
    ident_d = din("ident", (128, 128), f16)
    ccol_d = din("const_col", (1, 128, 1))
    crow_d = din("const_row", (2, 1, 128))
    wpred_d = din("wpred", (128, KT, ACT))
    bpred_d = din("bpred", (ACT, 1))
    padm_d = din("padm", (128, ST)) if pad_masked else None
    out_d = nc.dram_tensor("out", (ACT, S), f32, kind="ExternalOutput").ap()
    if dbg:
        def dout(name, shape, dt=f32):
            return nc.dram_tensor(name, shape, dt, kind="ExternalOutput").ap()
        stk_d = dout("dbg_stk", (KT, 128, S))
        h0_d = dout("dbg_h0", (KT, 128, S))
        hn0_d = dout("dbg_hn0", (KT, 128, S), f16)
        v0_d = dout("dbg_v0", (ST, 128, NH * (HD + 1)), f16)
        qt0_d = dout("dbg_qt0", (128, S), f16)
        kt0_d = dout("dbg_kt0", (128, S), f16)
        att0_d = dout("dbg_att0", (128, ST * CH), f16)
        yt0_d = dout("dbg_yt0", (KT, 128, S), f16)
        hp0_d = dout("dbg_hp0", (KT, 128, S))
        hL0_d = dout("dbg_hL0", (KT, 128, S))

    with tile.TileContext(nc) as tc, ExitStack() as ctx:
        ctx.enter_context(
            nc.allow_low_precision(reason="f32r/f16 storage; psum accum is f32"))
        const = ctx.enter_context(tc.tile_pool(name="const", bufs=1))
        resid = ctx.enter_context(tc.tile_pool(name="resid", bufs=1))
        hnp = ctx.enter_context(tc.tile_pool(name="hnp", bufs=1))
        vpool = ctx.enter_context(tc.tile_pool(name="vpool", bufs=1))
        ytp = ctx.enter_context(tc.tile_pool(name="ytp", bufs=1))
        qkp = ctx.enter_context(tc.tile_pool(name="qkp", bufs=1))
        attp = ctx.enter_context(tc.tile_pool(name="attp", bufs=3))
        wvp = ctx.enter_context(tc.tile_pool(name="wvp", bufs=1))
        wstr = ctx.enter_context(tc.tile_pool(name="wstr", bufs=2))
        wgrp = ctx.enter_context(tc.tile_pool(name="wgrp", bufs=2))
        bstr = ctx.enter_context(tc.tile_pool(name="bstr", bufs=2))
        tmp = ctx.enter_context(tc.tile_pool(name="tmp", bufs=3))
        stats = ctx.enter_context(tc.tile_pool(name="stats", bufs=3))
        apool = ctx.enter_context(tc.tile_pool(name="apool", bufs=2))
        psp = ctx.enter_context(tc.tile_pool(name="psp", bufs=1, space="PSUM"))

        def ps1(p=128, n=CH):
            return psp.tile([p, n], f32, name="b1", tag="b1", bufs=4)

        def ps2():
            return psp.tile([128, 2, 512], f32, name="b2", tag="b2", bufs=2)

        ones_col = const.tile([128, 1], f32r, name="ones_col", tag="ones_col")
        nc.sync.dma_start(ones_col[:], ccol_d[0].bitcast(f32r))
        ones_row = const.tile([1, 128], f32r, name="ones_row", tag="ones_row")
        nc.sync.dma_start(ones_row[:], crow_d[0].bitcast(f32r))
        h768_row = const.tile([1, 128], f32r, name="h768_row", tag="h768_row")
        nc.sync.dma_start(h768_row[:], crow_d[1].bitcast(f32r))
        masks_sb = const.tile([128, 3, CH], f16, name="masks", tag="masks")
        nc.sync.dma_start(masks_sb[:], masks_d)
        ident_sb = const.tile([128, 128], f16, name="ident", tag="ident")
        nc.sync.dma_start(ident_sb[:], ident_d)
        g_sb = const.tile([128, KT], f32, name="g_sb", tag="g_sb")
        nc.sync.dma_start(g_sb[:], lnemb_g)
        gb_sb = const.tile([128, KT], f32, name="gb_sb", tag="gb_sb")
        nc.sync.dma_start(gb_sb[:], lnemb_b)
        wpred_sb = const.tile([128, KT, ACT], f32r, name="wpred", tag="wpred")
        nc.sync.dma_start(wpred_sb[:], wpred_d.bitcast(f32r))
        bpred_sb = const.tile([ACT, 1], f32, name="bpred", tag="bpred")
        nc.sync.dma_start(bpred_sb[:], bpred_d)
        eps_sb = const.tile([1, 1], f32, name="eps_sb", tag="eps_sb")
        nc.vector.memset(eps_sb[:], float(H) * float(H) * LN_EPS)
        if pad_masked:
            padm_sb = const.tile([128, ST], f32, name="padm", tag="padm")
            nc.sync.dma_start(padm_sb[:], padm_d)

        hT = [resid.tile([128, S], f32r, name=f"h{k}", tag=f"h{k}") for k in range(KT)]
        hn = [hnp.tile([128, S], f16, name=f"hn{k}", tag=f"hn{k}") for k in range(KT)]
        yt = [ytp.tile([128, S], f16, name=f"y{k}", tag=f"y{k}") for k in range(KT)]

        def chunk(c):
            return slice(c * CH, (c + 1) * CH)

        def layernorm_chunk(xs, outs, c, gain=None, gbias=None, fp8_copy=False):
            # stats for chunk c, then normalize chunk c; emitting per-chunk
            # lets chunk c's DVE/ACT tail overlap chunk c+1's PE stats work
            sl = chunk(c)
            ps_s = ps1(1)
            ps_q = ps1(1)
            for k in range(KT):
                nc.tensor.matmul(ps_s[:], ones_col[:], xs[k][:, sl],
                                 start=(k == 0), stop=(k == KT - 1))
            for k in range(KT):
                sq = tmp.tile([128, CH], f32r, name="tm", tag="tm")
                nc.scalar.activation(sq[:], xs[k][:, sl].bitcast(f32), AF.Square)
                nc.tensor.matmul(ps_q[:], ones_col[:], sq[:],
                                 start=(k == 0), stop=(k == KT - 1))
            sums = stats.tile([1, CH], f32, name="st", tag="st")
            nc.vector.tensor_copy(sums[:], ps_s[:])
            t0 = stats.tile([1, CH], f32, name="st", tag="st")
            nc.vector.tensor_mul(t0[:], sums[:], sums[:])
            vx = stats.tile([1, CH], f32, name="st", tag="st")
            nc.vector.scalar_tensor_tensor(
                out=vx[:], in0=ps_q[:], scalar=float(H), in1=t0[:],
                op0=ALU.mult, op1=ALU.subtract)
            lnv = stats.tile([1, CH], f32, name="st", tag="st")
            nc.scalar.activation(lnv[:], vx[:], AF.Ln, bias=eps_sb[:])
            r_ = stats.tile([1, CH], f32r, name="rst", tag="rst", bufs=3)
            nc.scalar.activation(r_[:], lnv[:], AF.Exp, scale=-0.5)
            bv_ = stats.tile([1, CH], f32r, name="rst", tag="rst", bufs=3)
            nc.vector.scalar_tensor_tensor(
                out=bv_[:], in0=sums[:], scalar=-1.0, in1=r_[:],
                op0=ALU.mult, op1=ALU.mult)
            pa = ps1()
            nc.tensor.matmul(pa[:], h768_row[:], r_[:], start=True, stop=True)
            pb = ps1()
            nc.tensor.matmul(pb[:], ones_row[:], bv_[:], start=True, stop=True)
            for k in range(KT):
                tm = tmp.tile([128, CH], f32, name="tm", tag="tm")
                nc.vector.tensor_mul(tm[:], xs[k][:, sl], pa[:])
                if gain is None:
                    nc.vector.tensor_add(outs[k][:, sl], tm[:], pb[:])
                else:
                    tm2 = tmp.tile([128, CH], f32, name="tm", tag="tm")
                    nc.vector.tensor_add(tm2[:], tm[:], pb[:])
                    nc.vector.tensor_scalar(
                        out=outs[k][:, sl], in0=tm2[:],
                        scalar1=gain[:, k:k + 1], scalar2=gbias[:, k:k + 1],
                        op0=ALU.mult, op1=ALU.add)

        def layernorm(xs, outs, gain=None, gbias=None, fp8_copy=False):
            for c in range(NCH):
                layernorm_chunk(xs, outs, c, gain=gain, gbias=gbias,
                                fp8_copy=fp8_copy)

        # ================= embedding =================
        # scratch tiles share the weight-group slots (dead after embedding)
        sc_a = wgrp.tile([128, 2304], f32r, name="w1g", tag="w1g")
        sc_b = wgrp.tile([128, 2304], f32r, name="w2g", tag="w2g")
        misc_l = sc_a[0:MISC_K, 0:H]
        nc.sync.dma_start(misc_l, misc_lhs.bitcast(f32r))
        mis_l = sc_a[0:TEXT, H:H + H]
        nc.sync.dma_start(mis_l, mis_lhs.bitcast(f32r))
        misc_r = sc_b[0:MISC_K, 0:S]
        nc.sync.dma_start(misc_r, misc_rhs.bitcast(f32r))
        mis_r = sc_b[0:TEXT, S:2 * S]
        nc.sync.dma_start(mis_r, mis_rhs.bitcast(f32r))
        trhs = const.tile([128, 8, T], f16, name="trhs", tag="trhs")
        nc.sync.dma_start(trhs[:], time_rhs.rearrange("kb p t -> p kb t"))

        stk = [hnp.tile([128, S], f32r, name=f"stk{k}", tag=f"hn{k}")
               for k in range(KT)]  # shares the hn slots
        for k in range(KT):
            fsl = slice(k * 128, (k + 1) * 128)
            tl = wstr.tile([128, 8, 128], f16, name="wq", tag="wq")
            nc.sync.dma_start(tl[:], time_lhs[k].rearrange("kb p c -> p kb c"))
            ptime = ps1(128, T)
            for kb in range(8):
                nc.tensor.matmul(ptime[:], tl[:, kb, :], trhs[:, kb, :],
                                 start=(kb == 0), stop=(kb == 7))
            tsb = tmp.tile([128, T], f32, name="tm", tag="tm")
            nc.vector.tensor_copy(tsb[:], ptime[:])
            for c in range(NCH):
                sl = chunk(c)
                pstk = ps1()
                nc.tensor.matmul(pstk[:], misc_l[:, fsl], misc_r[:, sl],
                                 start=True, stop=False)
                nc.tensor.matmul(pstk[:], mis_l[:, fsl], mis_r[:, sl],
                                 start=False, stop=True)
                nc.vector.tensor_copy(stk[k][:, sl], pstk[:])
            stk_v = stk[k][:].rearrange("p (t a) -> p t a", a=TOTAL)
            nc.vector.tensor_tensor(out=stk_v, in0=stk_v,
                                    in1=bcast_free(tsb[:], TOTAL), op=ALU.add)
        if dbg:
            for k in range(KT):
                nc.sync.dma_start(stk_d[k].bitcast(f32r), stk[k][:])
        if emb_triv:
            # ln_emb gamma==1/beta==0: use the cheaper no-gain path, and
            # LN(LN(x)) == LN(x) to within eps/2 (~5e-6), so layer 0's
            # attention layernorm collapses to a cast copy
            layernorm(stk, hT)
            for k in range(KT):
                nc.vector.tensor_copy(hn[k][:], hT[k][:])
        else:
            layernorm(stk, hT, gain=g_sb, gbias=gb_sb)
            layernorm(hT, hn)   # LN1 for layer 0
        if dbg:
            for k in range(KT):
                nc.sync.dma_start(h0_d[k].bitcast(f32r), hT[k][:])

        # ================= layers =================
        for l in range(L):
            if dbg and l == 0:
                for k in range(KT):
                    nc.sync.dma_start(hn0_d[k], hn[k][:])
            bq_sb = bstr.tile([128, KT], f32, name="bq", tag="bq")
            nc.sync.dma_start(bq_sb[:], bq_d[l])
            bk_sb = bstr.tile([128, KT], f32, name="bk", tag="bk")
            nc.sync.dma_start(bk_sb[:], bk_d[l])
            bp_sb = bstr.tile([128, KT], f32, name="bp", tag="bp")
            nc.sync.dma_start(bp_sb[:], bp_d[l])
            b1_sb = bstr.tile([128, FT], f32, name="b1", tag="b1")
            nc.sync.dma_start(b1_sb[:], b1_d[l])
            b2_sb = bstr.tile([128, KT], f32, name="b2", tag="b2")
            nc.sync.dma_start(b2_sb[:], b2_d[l])
            bvb = qkp.tile([128, H], f32, name="bvb", tag="bvb")
            nc.sync.dma_start(bvb[:], bass.AP(tensor=bv_d.tensor,
                                              offset=bv_d[l].offset,
                                              ap=[[0, 128], [1, H]]))

            # ---- V (seq-major, fp16, ones column per head for softmax den) ----
            wv_sb = []
            for kb in range(KT):
                w_ = wvp.tile([128, H], f16, name=f"wvp{kb}", tag=f"wvp{kb}")
                nc.sync.dma_start(w_[:], wv_d[l, kb])
                wv_sb.append(w_)
            vt = []
            for si in range(ST):
                v_ = vpool.tile([128, NH, HD + 1], f16, name=f"v{si}", tag=f"v{si}")
                vt.append(v_)
                nc.vector.memset(v_[:, :, HD:HD + 1], 1.0)
            for si in range(ST):
                ssl = slice(si * 128, (si + 1) * 128)
                for c in range(2):
                    csl = slice(c * CH, (c + 1) * CH)
                    pv = ps1()
                    for kb in range(KT):
                        nc.tensor.matmul(pv[:], hn[kb][:, ssl], wv_sb[kb][:, csl],
                                         start=(kb == 0), stop=(kb == KT - 1))
                    dst = vt[si][:, c * KT:(c + 1) * KT, 0:HD]
                    nc.vector.tensor_tensor(
                        out=dst, in0=pv[:].rearrange("p (h d) -> p h d", d=HD),
                        in1=bvb[:, csl].rearrange("p (h d) -> p h d", d=HD),
                        op=ALU.add)

            if dbg and l == 0:
                for si in range(ST):
                    nc.sync.dma_start(
                        v0_d[si], vt[si][:].rearrange("p h d -> p (h d)"))

            # ---- attention: per-pair Q/K, then a software-pipelined stream of
            # (head, chunk) units: scores+exp now, AV one unit later,
            # normalize two units later (hides the reciprocal latency) ----
            wp_sb = []
            for fo in range(KT):
                w_ = wvp.tile([128, KT, 128], f16, name=f"wvp{fo}", tag=f"wvp{fo}")
                wp_sb.append(w_)
            qkt = {}

            def emit_qk(p):
                wq_sb = wstr.tile([128, KT, 128], f16, name="wq", tag="wq")
                nc.sync.dma_start(
                    wq_sb[:],
                    wq_d[l, p].rearrange("kb p c -> p kb c"))
                wk_sb = wstr.tile([128, KT, 128], f16, name="wk", tag="wk")
                nc.sync.dma_start(
                    wk_sb[:],
                    wk_d[l, p].rearrange("kb p c -> p kb c"))
                qt = qkp.tile([128, S], f16, name="qt", tag="qt")
                kt = qkp.tile([128, S], f16, name="kt", tag="kt")
                for c in range(NCH):
                    sl = chunk(c)
                    pq = ps1()
                    for kb in range(KT):
                        nc.tensor.matmul(pq[:], wq_sb[:, kb, :], hn[kb][:, sl],
                                         start=(kb == 0), stop=(kb == KT - 1))
                    nc.vector.tensor_scalar_add(out=qt[:, sl], in0=pq[:],
                                                scalar1=bq_sb[:, p:p + 1])
                    pk = ps1()
                    for kb in range(KT):
                        nc.tensor.matmul(pk[:], wk_sb[:, kb, :], hn[kb][:, sl],
                                         start=(kb == 0), stop=(kb == KT - 1))
                    nc.vector.tensor_scalar_add(out=kt[:, sl], in0=pk[:],
                                                scalar1=bk_sb[:, p:p + 1])
                if dbg and l == 0 and p == 0:
                    nc.sync.dma_start(qt0_d, qt[:])
                    nc.sync.dma_start(kt0_d, kt[:])
                # Wp column block for this pair (reuses the wv slots)
                nc.sync.dma_start(wp_sb[p][:],
                                  wp_d[l, p].rearrange("kb p c -> p kb c"))
                qkt[p] = (qt, kt)

            def emit_scores(u):
                p, hi, c = u["p"], u["hi"], u["c"]
                qt, kt = qkt[p]
                hsl = slice(hi * HD, (hi + 1) * HD)
                sl = chunk(c)
                nsk = 3 * (c + 1)
                att = attp.tile([128, ST, CH], f16, name="att", tag="att")
                for gs in range(0, nsk, 2):
                    ge = min(gs + 2, nsk)
                    psc = ps2()
                    for i in range(gs, ge):
                        diag = i - 3 * c
                        nc.tensor.matmul(psc[:, i - gs, 0:CH],
                                         kt[hsl, i * 128:(i + 1) * 128],
                                         qt[hsl, sl], start=True,
                                         stop=(diag < 0))
                        if diag >= 0:
                            # causal mask folded in on the PE: += -1e4
                            # above the diagonal (keeps exp->AV off the DVE)
                            nc.tensor.matmul(psc[:, i - gs, 0:CH],
                                             ident_sb[:],
                                             masks_sb[:, diag, :],
                                             start=False, stop=True)
                    if pad_masked:
                        for i in range(gs, ge):
                            nc.scalar.activation(att[:, i, :],
                                                 psc[:, i - gs, 0:CH],
                                                 AF.Exp, scale=0.125,
                                                 bias=padm_sb[:, i:i + 1])
                    else:
                        nc.scalar.activation(att[:, gs:ge, :],
                                             psc[:, 0:ge - gs, 0:CH],
                                             AF.Exp, scale=0.125)
                u["att"], u["hsl"], u["sl"], u["nsk"], u["h"] = \
                    att, hsl, sl, nsk, 2 * p + hi
                if dbg and l == 0 and u["h"] == 0 and c == 2:
                    nc.sync.dma_start(att0_d,
                                      att[:].rearrange("p i c -> p (i c)"))

            def emit_av(u):
                # AV into PSUM, then immediately evacuate: unnormalized Y
                # (f16) into yt, denominator row into the head's den batch.
                pY = ps1(HD + 1)
                for i in range(u["nsk"]):
                    nc.tensor.matmul(pY[:], vt[i][:, u["h"], :],
                                     u["att"][:, i, :],
                                     start=(i == 0), stop=(i == u["nsk"] - 1))
                nc.vector.tensor_copy(yt[u["p"]][u["hsl"], u["sl"]],
                                      pY[0:HD, :])
                nc.vector.tensor_copy(
                    u["den"][0:1, u["c"] * CH:(u["c"] + 1) * CH],
                    pY[HD:HD + 1, :])

            def emit_recip(units):
                # one Ln+Exp over the head's 3 chunk-denominators (batched
                # ACT; ln/exp share a table set so no ACT_TABLE_LOADs)
                den = units[0]["den"]
                lnd = stats.tile([1, S], f32, name="lnd", tag="den", bufs=2)
                nc.scalar.activation(lnd[:], den[:].bitcast(f32), AF.Ln)
                rec = stats.tile([1, S], f32r, name="rec", tag="rec", bufs=2)
                nc.scalar.activation(rec[:], lnd[:], AF.Exp, scale=-1.0)
                for u in units:
                    u["rec"] = rec

            def emit_norm(u):
                # broadcast recip to all 128 partitions so the in-place
                # multiply's SBUF operands share a start partition
                base = u["hi"] * HD
                pB = ps1()
                nc.tensor.matmul(pB[:], ones_row[0:1, :],
                                 u["rec"][0:1, u["sl"]], start=True, stop=True)
                yv = yt[u["p"]][u["hsl"], u["sl"]]
                nc.vector.tensor_tensor(out=yv, in0=yv,
                                        in1=pB[base:base + HD, :], op=ALU.mult)

            av_q = []
            done_heads = []

            def flush_av():
                u = av_q.pop(0)
                emit_av(u)
                if u["p"] == KT - 1 and u["hi"] == 1:
                    # final head: per-chunk in-place recip so the layer's
                    # drain tail stays under the HAM re-throttle window
                    units = u["units"]
                    if u["c"] == 0:
                        units[0]["recx"] = stats.tile(
                            [1, S], f32r, name="rec", tag="rec", bufs=2)
                    rec = units[0]["recx"]
                    csl = slice(u["c"] * CH, (u["c"] + 1) * CH)
                    nc.scalar.activation(rec[0:1, csl],
                                         u["den"][0:1, csl].bitcast(f32),
                                         AF.Ln)
                    nc.scalar.activation(rec[0:1, csl],
                                         rec[0:1, csl].bitcast(f32),
                                         AF.Exp, scale=-1.0)
                    u["rec"] = rec
                    if u["c"] == NCH - 1:
                        done_heads.append(units)
                        if len(done_heads) > 1:
                            for v in done_heads.pop(0):
                                emit_norm(v)
                elif u["c"] == NCH - 1:
                    # head complete: recip it, norm the head before it
                    # (PE stays busy on newer heads' score matmuls)
                    emit_recip(u["units"])
                    done_heads.append(u["units"])
                    if len(done_heads) > 1:
                        for v in done_heads.pop(0):
                            emit_norm(v)

            for p in range(KT):
                emit_qk(p)
                for hi in range(2):
                    den = stats.tile([1, S], f32r, name="den", tag="den",
                                     bufs=2)
                    units = []
                    for c in range(NCH):
                        u = {"p": p, "hi": hi, "c": c, "den": den,
                             "units": units}
                        emit_scores(u)
                        units.append(u)
                        av_q.append(u)
                        if len(av_q) > 2:
                            flush_av()
            while av_q:
                flush_av()
            for units in done_heads:
                for v in units:
                    emit_norm(v)

            if dbg and l == 0:
                for k in range(KT):
                    nc.sync.dma_start(yt0_d[k], yt[k][:])
            # ---- proj + residual, with the pre-MLP layernorm interleaved
            # per chunk (chunk c's LN DVE tail overlaps chunk c+1's proj MMs)
            for c in range(NCH):
                sl = chunk(c)
                for fo in range(KT):
                    pp = ps1()
                    for kb in range(KT):
                        nc.tensor.matmul(pp[:], wp_sb[fo][:, kb, :], yt[kb][:, sl],
                                         start=(kb == 0), stop=(kb == KT - 1))
                    nc.vector.scalar_tensor_tensor(
                        out=hT[fo][:, sl], in0=pp[:], scalar=bp_sb[:, fo:fo + 1],
                        in1=hT[fo][:, sl], op0=ALU.add, op1=ALU.add)
                layernorm_chunk(hT, hn, c)

            if dbg and l == 0:
                for k in range(KT):
                    nc.sync.dma_start(hp0_d[k].bitcast(f32r), hT[k][:])
            # ---- MLP (stream W1/W2 in ff-groups, accumulate into hT) ----
            for g in range(FT // MG):
                w1g = wgrp.tile([128, MG, KT, 128], f16, name="w1g", tag="w1g")
                nc.sync.dma_start(
                    w1g[:],
                    w1_d[l, g].rearrange("f kb p c -> p f kb c"))
                w2g = wgrp.tile([128, MG, KT, 128], f16, name="w2g", tag="w2g")
                nc.sync.dma_start(
                    w2g[:],
                    w2_d[l, g].rearrange("f kb p c -> p f kb c"))
                for c in range(NCH):
                    sl = chunk(c)
                    py_a = ps2()
                    py_b = ps2()
                    py_c = ps1()
                    py_d = ps1()

                    def ypsum(fo):
                        if fo < 2:
                            return py_a[:, fo, 0:CH]
                        if fo < 4:
                            return py_b[:, fo - 2, 0:CH]
                        return (py_c if fo == 4 else py_d)[:]

                    for f in range(MG):
                        ff = g * MG + f
                        pa = ps1()
                        for kb in range(KT):
                            nc.tensor.matmul(pa[:], w1g[:, f, kb, :],
                                             hn[kb][:, sl],
                                             start=(kb == 0), stop=(kb == KT - 1))
                        aT = apool.tile([128, CH], f16, name="aT", tag="aT")
                        nc.scalar.activation(aT[:], pa[:], AF.Gelu,
                                             bias=b1_sb[:, ff:ff + 1])
                        for fo in range(KT):
                            nc.tensor.matmul(ypsum(fo), w2g[:, f, fo, :], aT[:],
                                             start=(f == 0), stop=(f == MG - 1))
                    last = (g == FT // MG - 1)
                    for fo in range(KT):
                        if last:
                            nc.vector.scalar_tensor_tensor(
                                out=hT[fo][:, sl], in0=ypsum(fo),
                                scalar=b2_sb[:, fo:fo + 1], in1=hT[fo][:, sl],
                                op0=ALU.add, op1=ALU.add)
                        else:
                            nc.vector.tensor_add(hT[fo][:, sl], ypsum(fo),
                                                 hT[fo][:, sl])
                    if last and l < L - 1:
                        # next layer's attention layernorm, chunk-interleaved
                        # into the last MLP group so hn chunk 0 is ready the
                        # moment the MLP matmuls finish (keeps PE warm into
                        # the next layer's V matmuls)
                        layernorm_chunk(hT, hn, c)
            if dbg and l == 0:
                for k in range(KT):
                    nc.sync.dma_start(hL0_d[k].bitcast(f32r), hT[k][:])
        # ================= prediction head =================
        pred_sb = hnp.tile([ACT, S], f32, name="hn0", tag="hn0")
        for c in range(NCH):
            sl = chunk(c)
            pp = ps1(ACT)
            for kb in range(KT):
                nc.tensor.matmul(pp[:], wpred_sb[:, kb, :], hT[kb][:, sl],
                                 start=(kb == 0), stop=(kb == KT - 1))
            nc.vector.tensor_scalar_add(out=pred_sb[:, sl], in0=pp[:],
                                        scalar1=bpred_sb[:])
        nc.sync.dma_start(out_d, pred_sb[:])

    nc.compile()
    return nc


# ---------------- host-side preparation ----------------

def _tile_lhs(w):
    """(Hin, Hout) -> [fo, kb, 128, 128] with [fo,kb,p,c] = w[128kb+p, 128fo+c]."""
    hin, hout = w.shape
    return np.ascontiguousarray(
        w.reshape(hin // 128, 128, hout // 128, 128).transpose(2, 0, 1, 3))


def prep_shared(inp):
    """Weight-side arrays, identical for every core."""
    d = {}
    g1, b1l = inp["ln1_g"], inp["ln1_b"]
    g2, b2l = inp["ln2_g"], inp["ln2_b"]
    wq = inp["Wq"] * g1[:, :, None]
    wk = inp["Wk"] * g1[:, :, None]
    wv = inp["Wv"] * g1[:, :, None]
    w1 = inp["W1"] * g2[:, :, None]
    bq = inp["bq"] + np.einsum("lh,lho->lo", b1l, inp["Wq"])
    bk = inp["bk"] + np.einsum("lh,lho->lo", b1l, inp["Wk"])
    bv = inp["bv"] + np.einsum("lh,lho->lo", b1l, inp["Wv"])
    b1 = inp["b1"] + np.einsum("lh,lho->lo", b2l, inp["W1"])
    d["wq"] = np.stack([_tile_lhs(wq[l]) for l in range(L)]).astype(np.float16)
    d["wk"] = np.stack([_tile_lhs(wk[l]) for l in range(L)]).astype(np.float16)
    d["wp"] = np.stack([_tile_lhs(inp["Wp"][l]) for l in range(L)]).astype(np.float16)
    d["wv"] = np.stack([wv[l].reshape(KT, 128, H) for l in range(L)]).astype(np.float16)
    w1t = np.stack([_tile_lhs(w1[l]) for l in range(L)])          # (L,24,6,128,128)
    d["w1"] = w1t.reshape(L, FT // MG, MG, KT, 128, 128).astype(np.float16)
    w2t = np.stack([_tile_lhs(inp["W2"][l]) for l in range(L)])   # (L,6,24,128,128)
    d["w2"] = np.ascontiguousarray(w2t.transpose(0, 2, 1, 3, 4)).reshape(
        L, FT // MG, MG, KT, 128, 128).astype(np.float16)
    d["bq"] = np.ascontiguousarray(bq.reshape(L, KT, 128).transpose(0, 2, 1))
    d["bk"] = np.ascontiguousarray(bk.reshape(L, KT, 128).transpose(0, 2, 1))
    d["bv"] = bv.reshape(L, 1, H)
    d["bp"] = np.ascontiguousarray(inp["bp"].reshape(L, KT, 128).transpose(0, 2, 1))
    d["b1"] = np.ascontiguousarray(b1.reshape(L, FT, 128).transpose(0, 2, 1))
    d["b2"] = np.ascontiguousarray(inp["b2"].reshape(L, KT, 128).transpose(0, 2, 1))
    # embedding misc lhs: [W_img | W_act | W_ret | b_img;b_act;b_ret | W_pos[:16]]
    d["misc_lhs"] = np.concatenate([
        inp["W_img"], inp["W_act"], inp["W_ret"],
        inp["b_img"][None], inp["b_act"][None], inp["b_ret"][None],
        inp["W_pos"][:STATE_TOK]], axis=0).astype(np.float32)
    d["mis_lhs"] = inp["W_mis"].astype(np.float32)
    d["time_lhs"] = _tile_lhs(inp["W_time"]).astype(np.float16)
    d["lnemb_g"] = np.ascontiguousarray(
        inp["ln_emb_g"].reshape(KT, 128).T).astype(np.float32)
    d["lnemb_b"] = np.ascontiguousarray(
        inp["ln_emb_b"].reshape(KT, 128).T).astype(np.float32)
    r = np.arange(128)[:, None]
    cc = np.arange(CH)[None, :]
    d["masks"] = np.stack([(cc < 128 * dd + r) * NEG for dd in range(3)],
                          axis=1).astype(np.float16)
    d["ident"] = np.eye(128, dtype=np.float16)
    d["wpred"] = np.ascontiguousarray(
        inp["W_pred"].reshape(KT, 128, ACT).transpose(1, 0, 2)).astype(np.float32)
    d["bpred"] = inp["b_pred"].reshape(ACT, 1).astype(np.float32)
    d["const_col"] = np.ones((1, 128, 1), np.float32)
    d["const_row"] = np.stack([np.ones((1, 128), np.float32),
                               np.full((1, 128), float(H), np.float32)])
    return {k: np.ascontiguousarray(v) for k, v in d.items()}


def prep_core(inp, b, pad_masked):
    """Per-core (per batch element) data tensors."""
    d = {}
    misc_rhs = np.zeros((MISC_K, S), np.float32)
    mis_rhs = np.zeros((TEXT, S), np.float32)
    timeoh = np.zeros((MAX_EP, T), np.float32)
    img = np.asarray(inp["images"][b], np.float32).reshape(T, IMG_DIM, IMG_CH)
    act = np.asarray(inp["actions"][b], np.float32)
    rtg = np.asarray(inp["returns_to_go"][b], np.float32)
    mis = np.asarray(inp["missions"][b])
    ts = np.asarray(inp["timesteps"][b])
    for t in range(T):
        base = TOTAL * t
        for c in range(IMG_CH):
            misc_rhs[0:IMG_DIM, base + 1 + c] = img[t, :, c]
        misc_rhs[IMG_DIM:IMG_DIM + ACT, base + TOTAL - 1] = act[t]
        misc_rhs[IMG_DIM + ACT, base] = rtg[t, 0]
        misc_rhs[IMG_DIM + ACT + 1, base + 1:base + 1 + IMG_CH] = 1.0
        misc_rhs[IMG_DIM + ACT + 2, base + TOTAL - 1] = 1.0
        misc_rhs[IMG_DIM + ACT + 3, base] = 1.0
        for j in range(STATE_TOK):
            misc_rhs[IMG_DIM + ACT + 4 + j, base + 1 + j] = 1.0
        for j in range(M):
            mis_rhs[mis[t, j], base + 1 + IMG_CH + j] = 1.0
        timeoh[ts[t], t] = 1.0
    d["misc_rhs"] = misc_rhs
    d["mis_rhs"] = mis_rhs
    d["time_rhs"] = timeoh.reshape(8, 128, T).astype(np.float16)
    if pad_masked:
        am = np.asarray(inp["attention_mask"][b], np.float32)
        mm = np.asarray(inp["mission_masks"][b], np.float32)
        tok = np.concatenate([np.ones((T, 1 + IMG_CH), np.float32), mm,
                              np.ones((T, 1), np.float32)], axis=1)
        m = (tok * am[:, None]).reshape(S)
        d["padm"] = np.ascontiguousarray(
            ((1.0 - m) * NEG).reshape(ST, 128).T).astype(np.float32)
    return d


_CACHE = {}


def _get_nc(pad_masked, b2nz=False, emb_triv=True):
    key = (pad_masked, b2nz, emb_triv)
    if key not in _CACHE:
        _CACHE[key] = build_kernel(pad_masked, b2nz, emb_triv)
    return _CACHE[key]


def kernel(**inputs):
    pad_masked = not (
        np.all(np.asarray(inputs["mission_masks"]) == 1.0)
        and np.all(np.asarray(inputs["attention_mask"]) == 1))
    b2nz = bool(np.any(np.asarray(inputs["b2"]) != 0))
    emb_triv = bool(np.all(np.asarray(inputs["ln_emb_g"]) == 1.0)
                    and np.all(np.asarray(inputs["ln_emb_b"]) == 0.0))
    nc = _get_nc(pad_masked, b2nz, emb_triv)
    shared = prep_shared({k: np.asarray(v) for k, v in inputs.items()})
    in_maps = []
    for b in range(N_CORES):
        m = dict(shared)
        m.update(prep_core(inputs, b, pad_masked))
        in_maps.append(m)
    res = bass_utils.run_bass_kernel_spmd(nc, in_maps, core_ids=list(range(N_CORES)))
    # gather: out[b, t] = predT_b[:, 18*t + valid_t + 3]
    mm = np.asarray(inputs["mission_masks"])
    out = np.zeros((B, T, ACT), np.float32)
    tt = np.arange(T)
    for b in range(B):
        gcol = TOTAL * tt + (mm[b].sum(axis=1) + IMG_CH).astype(np.int64)
        out[b] = np.asarray(res.results[b]["out"], np.float32)[:, gcol].T
    return out

